# revision 31
# baseline (speedup 1.0000x reference)
"""Two-layer LSTM (linear cell/output activations) + FC head on 8 NeuronCores.

Strategy (data-parallel over batch, per the sharding hint):
  - B=32 split across 8 cores -> B_local=4 per core; weights replicated.
  - State transposed: h^T/c^T are [H on partitions, (k,b) on free]; the
    per-step recurrence is z^T += U^T @ h^T with U stationary, landing as
    [128, 4] column blocks in PSUM.
  - z lives ENTIRELY in PSUM: the per-chunk input projections (x@W0, h0@W1)
    and rank-1 bias matmuls write xw+b into PSUM up front; the per-step
    recurrence matmuls accumulate U@h on top (start=False). No DVE add.
  - PSUM layout: 4 banks per layer, bank = t%4 (parity). The engine reading
    z for step t (ACT) touches bank t%4 while the PE writes bank (t+1)%4 —
    never the same bank, so Tile's bank tracker keeps them parallel.
  - The two layers are software-pipelined ONE CHUNK apart: the loop body
    interleaves L0 step s of chunk i with L1 step s of chunk i-1. Each
    layer's sigmoid+gate-math chain hides under the other layer's matmul
    block. Prologue runs L0 chunk 0; epilogue runs L1 chunk 31 + FC.
  - Per-step chain: ACT copies g to SBUF + sigmoids i,f (early, after their
    m-blocks) and o; DVE does 4 small SBUF-only tensor_tensor ops. PE order
    per step: g-blocks, i, f, o so the sigmoid can start at ~75% of the
    matmul block.
"""

import os
import numpy as np
from contextlib import ExitStack

os.environ.setdefault("MYCRO_LOCAL_CACHE", "1")

B, T, I, H, O = 32, 2048, 128, 256, 128
NCORES = 8
BL = B // NCORES          # 4 batch elements per core
CHUNK = 64                # timesteps per loop body
G4 = 4 * H                # 1024 gate columns
NM = G4 // 128            # 8 gate m-blocks of 128
KT = H // 128             # 2 contraction tiles
NP = 4                    # PSUM bank parity groups
TG = CHUNK // NP          # 16 step-groups per parity
BKCOLS = TG * NM * BL     # 512 fp32 cols per bank

MM_BF16 = True

_cache = {}


def _np_mmdt():
    if MM_BF16:
        import ml_dtypes
        return ml_dtypes.bfloat16
    return np.float32


def _build(tiny=False):
    import concourse.bacc as bacc
    import concourse.bass as bass
    import concourse.tile as tile
    import concourse.mybir as mybir

    f32 = mybir.dt.float32
    mdt = mybir.dt.bfloat16 if MM_BF16 else f32
    AF = mybir.ActivationFunctionType
    ALU = mybir.AluOpType

    nc = bacc.Bacc("TRN2", target_bir_lowering=False, debug=False,
                   num_devices=NCORES)

    xprep_d = nc.declare_dram_parameter("xprep", [I, BL, T], mdt, isOutput=False)
    w0_d = nc.declare_dram_parameter("w0", [I, G4], mdt, isOutput=False)
    u0_d = nc.declare_dram_parameter("u0", [H, G4], mdt, isOutput=False)
    w1_d = nc.declare_dram_parameter("w1", [H, G4], mdt, isOutput=False)
    u1_d = nc.declare_dram_parameter("u1", [H, G4], mdt, isOutput=False)
    wfc_d = nc.declare_dram_parameter("wfc", [H, O], mdt, isOutput=False)
    b0r_d = nc.declare_dram_parameter("b0r", [128, 128], mdt, isOutput=False)
    b1r_d = nc.declare_dram_parameter("b1r", [128, 128], mdt, isOutput=False)
    oneh_d = nc.declare_dram_parameter("oneh", [128, BKCOLS], mdt,
                                       isOutput=False)
    bfct_d = nc.declare_dram_parameter("bfct", [128, 1], f32, isOutput=False)
    out_d = nc.declare_dram_parameter("outT", [O, BL], f32, isOutput=True)

    with tile.TileContext(nc) as tc, ExitStack() as ctx:
        if tiny:
            pool = ctx.enter_context(tc.tile_pool(name="tp", bufs=1))
            t1 = pool.tile([128, BL], mdt, tag="t1")
            t2 = pool.tile([128, BL], f32, tag="t2")
            nc.sync.dma_start(t1[:, :], xprep_d[:, :, 0])
            nc.vector.tensor_copy(t2[:, :], t1[:, :])
            nc.sync.dma_start(out_d[:, :], t2[:, :])
            nc.compile()
            return nc

        const = ctx.enter_context(tc.tile_pool(name="const", bufs=1))
        work = ctx.enter_context(tc.tile_pool(name="work", bufs=4))
        # single-slot pool for ig: the WAR on the shared slot forces the
        # scheduler to finish stream A's c-update before starting stream
        # B's ig, keeping the DVE FIFO round-robin between the two layers.
        igp = ctx.enter_context(tc.tile_pool(name="igp", bufs=1))
        psum = ctx.enter_context(tc.tile_pool(name="psum", bufs=1, space="PSUM"))

        # ---- persistent SBUF residents ----
        # xall col = b*T + t
        xall = const.tile([128, BL * T], mdt, tag="xall")
        w0 = const.tile([128, G4], mdt, tag="w0")
        u0 = [const.tile([128, G4], mdt, tag=f"u0_{k}", name=f"u0_{k}")
              for k in range(KT)]
        w1 = [const.tile([128, G4], mdt, tag=f"w1_{k}", name=f"w1_{k}")
              for k in range(KT)]
        u1 = [const.tile([128, G4], mdt, tag=f"u1_{k}", name=f"u1_{k}")
              for k in range(KT)]
        wf = [const.tile([128, O], mdt, tag=f"wf_{k}", name=f"wf_{k}")
              for k in range(KT)]
        # bias matrices for the rank-1 bias matmuls: row r (partition r) =
        # bias of m-block r; block-one-hot rhs maps row m to col block m,
        # so ONE matmul adds every m-block's bias to a whole PSUM bank.
        ball0 = const.tile([128, 128], mdt, tag="ball0")
        ball1 = const.tile([128, 128], mdt, tag="ball1")
        onehot = const.tile([128, BKCOLS], mdt, tag="onehot")
        bfct = const.tile([128, 1], f32, tag="bfct")
        # h0 stream for one chunk, parity-major: col = k*256 + (s%4)*64
        #   + (s//4)*4 + b
        h0t = const.tile([128, KT * CHUNK * BL], mdt, tag="h0t")
        # recurrent state czg = [c (8) | zg staging (8)], col = k*BL + b;
        # keeping c and zg adjacent lets ONE tensor_tensor compute both
        # f*c and i*zg (gate layout is f,i,o,g).
        czg0 = const.tile([128, 2 * KT * BL], f32, tag="czg0")
        czg1 = const.tile([128, 2 * KT * BL], f32, tag="czg1")
        h1 = const.tile([128, KT * BL], mdt, tag="h1")

        # ---- PSUM: z for each layer, 4 banks, bank = step parity ----
        # col = (s%4)*BKCOLS + m*TG*BL + (s//4)*BL + b
        zl0 = psum.tile([128, NP * BKCOLS], f32, tag="zl0")
        zl1 = psum.tile([128, NP * BKCOLS], f32, tag="zl1")

        nc.sync.dma_start(xall[:, :].rearrange("p (b t) -> p b t", b=BL),
                          xprep_d[:, :, :])
        nc.sync.dma_start(w0[:, :], w0_d[:, :])
        for k in range(KT):
            sl = slice(k * 128, (k + 1) * 128)
            nc.sync.dma_start(u0[k][:, :], u0_d[sl, :])
            nc.sync.dma_start(w1[k][:, :], w1_d[sl, :])
            nc.sync.dma_start(u1[k][:, :], u1_d[sl, :])
            nc.sync.dma_start(wf[k][:, :], wfc_d[sl, :])
        nc.sync.dma_start(ball0[:, :], b0r_d[:, :])
        nc.sync.dma_start(ball1[:, :], b1r_d[:, :])
        nc.sync.dma_start(bfct[:, :], bfct_d[:, :])
        nc.sync.dma_start(onehot[:, :], oneh_d[:, :])
        nc.vector.memset(h0t[:, :], 0.0)
        nc.vector.memset(czg0[:, :], 0.0)
        nc.vector.memset(czg1[:, :], 0.0)
        nc.vector.memset(h1[:, :], 0.0)

        # z bank layout is tg-major: col = pr*BKCOLS + tg*(NM*BL) + m*BL + b
        # so one step's z is 32 CONTIGUOUS fp32 (i,f,o = 24, g = 8).
        # h0 layout: col = k*CHUNK*BL + pr*TG*BL + tg*BL + b
        h0v = h0t[:, :].rearrange("p (k pr tg b) -> p k pr tg b",
                                  k=KT, pr=NP, tg=TG)

        SW = NM * BL          # 32 z cols per step

        def zstep(zl, s):
            """offset of step s's z block (32 cols)."""
            return (s % NP) * BKCOLS + (s // NP) * SW

        def zcols(zl, s, m):
            """z AP for step s, m-block m: [128, BL]."""
            base = zstep(zl, s) + m * BL
            return zl[:, base:base + BL]

        def h0cols(s, k):
            return h0v[:, k, s % NP, s // NP]

        def h0ap_write(s):
            """[128, KT, BL] write AP for step s's h in h0t."""
            return h0v[:, :, s % NP, s // NP]

        # m-block order: g (6,7) first, then i (0,1), f (2,3), o (4,5)
        M_ORDER = [6, 7, 0, 1, 2, 3, 4, 5]

        def lstm_step(s, uw, zl, cst, h_rhs_fn, h_out_ap):
            S = KT * BL
            for m in M_ORDER:
                for k in range(KT):
                    nc.tensor.matmul(zcols(zl, s, m),
                                     lhsT=uw[k][:, m * 128:(m + 1) * 128],
                                     rhs=h_rhs_fn(k),
                                     start=False, stop=(k == KT - 1))
            zb = zstep(zl, s)
            # ACT sigmoids f,i,o while DVE stages g next to c (concurrent)
            sg = work.tile([128, 3 * S], f32, tag="sg")
            nc.scalar.activation(sg[:, :], zl[:, zb:zb + 3 * S], AF.Sigmoid)
            nc.vector.tensor_copy(cst[:, S:2 * S],
                                  zl[:, zb + 3 * S:zb + 4 * S])
            # DVE: [f*c | i*g] in one op; c = sum halves; h = o*c
            fcig = igp.tile([128, 2 * S], f32, tag="fcig")
            nc.vector.tensor_tensor(fcig[:, :], sg[:, 0:2 * S], cst[:, :],
                                    ALU.mult)
            nc.vector.tensor_tensor(cst[:, 0:S], fcig[:, 0:S],
                                    fcig[:, S:2 * S], ALU.add)
            nc.vector.tensor_tensor(
                h_out_ap,
                sg[:, 2 * S:3 * S].rearrange("p (k b) -> p k b", k=KT),
                cst[:, 0:S].rearrange("p (k b) -> p k b", k=KT), ALU.mult)

        def proj_bias(zl, ball):
            """z += bias: one N=512 matmul per bank adds every m-block's
            bias (ball row m) to its col block via the block-one-hot rhs."""
            for p in range(NP):
                nc.tensor.matmul(
                    zl[:, p * BKCOLS:(p + 1) * BKCOLS],
                    lhsT=ball[:, :], rhs=onehot[:, :],
                    start=False, stop=False)

        # proj-output view [p, pr, tg, m, b] (strided per m-block)
        zp0 = zl0[:, :].rearrange("p (pr tg m b) -> p pr tg m b",
                                  pr=NP, tg=TG, m=NM)
        zp1 = zl1[:, :].rearrange("p (pr tg m b) -> p pr tg m b",
                                  pr=NP, tg=TG, m=NM)

        def proj_l0(iv):
            """xw0 for chunk at t0=iv into zl0 (start=True clears banks)."""
            xq = work.tile([128, BL * CHUNK], mdt, tag="xq")
            # xq col = (t%4)*64 + (t//4)*4 + b  <- xall[b*T + iv + p + 4*tg]
            src = xall[:, :].rearrange("p (b t) -> p b t", b=BL)
            nc.vector.tensor_copy(
                xq[:, :].rearrange("p (pr tg b) -> p pr tg b", pr=NP, tg=TG),
                src[:, :, bass.ds(iv, CHUNK)].rearrange(
                    "p b (tg pr) -> p pr tg b", pr=NP),
            )
            for m in range(NM):
                for p in range(NP):
                    nc.tensor.matmul(
                        zp0[:, p, :, m],
                        lhsT=w0[:, m * 128:(m + 1) * 128],
                        rhs=xq[:, p * TG * BL:(p + 1) * TG * BL],
                        start=(m == 0), stop=False)
            proj_bias(zl0, ball0)

        def proj_l1():
            """xw1 = W1 @ h0(prev chunk) into zl1."""
            for m in range(NM):
                for p in range(NP):
                    for k in range(KT):
                        nc.tensor.matmul(
                            zp1[:, p, :, m],
                            lhsT=w1[k][:, m * 128:(m + 1) * 128],
                            rhs=h0t[:, k * CHUNK * BL + p * TG * BL:
                                    k * CHUNK * BL + (p + 1) * TG * BL],
                            start=(m == 0 and k == 0), stop=False)
            proj_bias(zl1, ball1)

        def l0_step(s):
            lstm_step(s, u0, zl0, czg0,
                      lambda k, _s=s: h0cols((_s - 1) % CHUNK, k),
                      h0ap_write(s))

        def l1_step(s):
            lstm_step(s, u1, zl1, czg1,
                      lambda k: h1[:, k * BL:(k + 1) * BL],
                      h1[:, :].rearrange("p (k b) -> p k b", k=KT))

        # ---- prologue: L0 chunk 0 ----
        proj_l0(0)
        for s in range(CHUNK):
            l0_step(s)

        # ---- main loop: L0 chunk i (t0=iv), L1 chunk i-1 ----
        from concourse.engine_type import EngineType
        with tc.For_i(CHUNK, T, CHUNK,
                      hint_engines=(EngineType.PE, EngineType.DVE,
                                    EngineType.Activation)) as iv:
            proj_l1()
            proj_l0(iv)
            for s in range(CHUNK):
                l0_step(s)
                l1_step(s)

        # ---- epilogue: L1 chunk 31, FC head ----
        proj_l1()
        for s in range(CHUNK):
            l1_step(s)

        psf = zl0[:, 0:BL]
        for k in range(KT):
            nc.tensor.matmul(psf, lhsT=wf[k][:, :],
                             rhs=h1[:, k * BL:(k + 1) * BL],
                             start=(k == 0), stop=(k == KT - 1))
        oT = work.tile([128, BL], f32, tag="oT")
        nc.scalar.activation(oT[:, :], psf, AF.Identity, bias=bfct[:, 0:1])
        nc.sync.dma_start(out_d[:, :], oT[:, :])

    nc.compile()
    return nc


def _get_compiled():
    if "main" not in _cache:
        _cache["main"] = _build()
    return _cache["main"]


def _ballmat(b, perm, mdt):
    m = np.zeros((128, 128), np.float32)
    m[0:NM, :] = np.asarray(b, np.float32)[perm].reshape(NM, 128)
    return np.ascontiguousarray(m.astype(mdt))


def _onehot(mdt):
    # z bank layout is tg-major: col = tg*(NM*BL) + m*BL + b -> row m hot
    # wherever (col % (NM*BL)) // BL == m
    m = np.zeros((128, BKCOLS), np.float32)
    cols = np.arange(BKCOLS)
    m[(cols % (NM * BL)) // BL, cols] = 1.0
    return np.ascontiguousarray(m.astype(mdt))


def _in_maps(input_seq, W0, U0, b0, W1, U1, b1, Wfc, bfc):
    mdt = _np_mmdt()
    x = np.asarray(input_seq, dtype=np.float32)
    # reorder gate blocks (i,f,g,o) -> (f,i,o,g)
    perm = np.concatenate([np.arange(H, 2 * H),
                           np.arange(0, H),
                           np.arange(3 * H, 4 * H),
                           np.arange(2 * H, 3 * H)])

    def gp(w):
        return np.ascontiguousarray(
            np.asarray(w, np.float32)[..., perm].astype(mdt))

    shared = {
        "w0": gp(W0),
        "u0": gp(U0),
        "w1": gp(W1),
        "u1": gp(U1),
        "wfc": np.ascontiguousarray(np.asarray(Wfc, np.float32).astype(mdt)),
        "b0r": _ballmat(b0, perm, mdt),
        "b1r": _ballmat(b1, perm, mdt),
        "oneh": _onehot(mdt),
        "bfct": np.ascontiguousarray(
            np.asarray(bfc, np.float32).reshape(1, 128).T),
    }
    in_maps = []
    for c in range(NCORES):
        xs = x[c * BL:(c + 1) * BL]                       # [BL, T, I]
        xp = np.ascontiguousarray(xs.transpose(2, 0, 1).astype(mdt))
        m = dict(shared)
        m["xprep"] = xp
        in_maps.append(m)
    return in_maps


def _assemble(res):
    out = np.empty((B, 1, O), np.float32)
    for c in range(NCORES):
        out[c * BL:(c + 1) * BL, 0, :] = res.results[c]["outT"].T
    return out


def _run(nc, inputs):
    from concourse.bass_utils import run_bass_kernel_spmd
    in_maps = _in_maps(**inputs)
    res = run_bass_kernel_spmd(nc, in_maps, list(range(NCORES)))
    return _assemble(res)


def kernel(input_seq, W0, U0, b0, W1, U1, b1, Wfc, bfc):
    nc = _get_compiled()
    return _run(nc, dict(input_seq=input_seq, W0=W0, U0=U0, b0=b0, W1=W1,
                         U1=U1, b1=b1, Wfc=Wfc, bfc=bfc))


# revision 32
# speedup vs baseline: 1.0736x; 1.0736x over previous
"""Two-layer LSTM (linear cell/output activations) + FC head on 8 NeuronCores.

Strategy (data-parallel over batch, per the sharding hint):
  - B=32 split across 8 cores -> B_local=4 per core; weights replicated.
  - State transposed: h^T/c^T are [H on partitions, (k,b) on free]; the
    per-step recurrence is z^T += U^T @ h^T with U stationary, landing as
    [128, 4] column blocks in PSUM.
  - z lives ENTIRELY in PSUM: the per-chunk input projections (x@W0, h0@W1)
    and rank-1 bias matmuls write xw+b into PSUM up front; the per-step
    recurrence matmuls accumulate U@h on top (start=False). No DVE add.
  - PSUM layout: 4 banks per layer, bank = t%4 (parity). The engine reading
    z for step t (ACT) touches bank t%4 while the PE writes bank (t+1)%4 —
    never the same bank, so Tile's bank tracker keeps them parallel.
  - The two layers are software-pipelined ONE CHUNK apart: the loop body
    interleaves L0 step s of chunk i with L1 step s of chunk i-1. Each
    layer's sigmoid+gate-math chain hides under the other layer's matmul
    block. Prologue runs L0 chunk 0; epilogue runs L1 chunk 31 + FC.
  - Per-step chain: ACT copies g to SBUF + sigmoids i,f (early, after their
    m-blocks) and o; DVE does 4 small SBUF-only tensor_tensor ops. PE order
    per step: g-blocks, i, f, o so the sigmoid can start at ~75% of the
    matmul block.
"""

import os
import numpy as np
from contextlib import ExitStack

os.environ.setdefault("MYCRO_LOCAL_CACHE", "1")

B, T, I, H, O = 32, 2048, 128, 256, 128
NCORES = 8
BL = B // NCORES          # 4 batch elements per core
CHUNK = 64                # timesteps per loop body
G4 = 4 * H                # 1024 gate columns
NM = G4 // 128            # 8 gate m-blocks of 128
KT = H // 128             # 2 contraction tiles
NP = 4                    # PSUM bank parity groups
TG = CHUNK // NP          # 16 step-groups per parity
BKCOLS = TG * NM * BL     # 512 fp32 cols per bank

MM_BF16 = True

_cache = {}


def _np_mmdt():
    if MM_BF16:
        import ml_dtypes
        return ml_dtypes.bfloat16
    return np.float32


def _build(tiny=False):
    import concourse.bacc as bacc
    import concourse.bass as bass
    import concourse.tile as tile
    import concourse.mybir as mybir

    f32 = mybir.dt.float32
    mdt = mybir.dt.bfloat16 if MM_BF16 else f32
    AF = mybir.ActivationFunctionType
    ALU = mybir.AluOpType

    nc = bacc.Bacc("TRN2", target_bir_lowering=False, debug=False,
                   num_devices=NCORES)

    xprep_d = nc.declare_dram_parameter("xprep", [I, BL, T], mdt, isOutput=False)
    w0_d = nc.declare_dram_parameter("w0", [I, G4], mdt, isOutput=False)
    u0_d = nc.declare_dram_parameter("u0", [H, G4], mdt, isOutput=False)
    w1_d = nc.declare_dram_parameter("w1", [H, G4], mdt, isOutput=False)
    u1_d = nc.declare_dram_parameter("u1", [H, G4], mdt, isOutput=False)
    wfc_d = nc.declare_dram_parameter("wfc", [H, O], mdt, isOutput=False)
    b0r_d = nc.declare_dram_parameter("b0r", [128, 128], mdt, isOutput=False)
    b1r_d = nc.declare_dram_parameter("b1r", [128, 128], mdt, isOutput=False)
    oneh_d = nc.declare_dram_parameter("oneh", [128, BKCOLS], mdt,
                                       isOutput=False)
    bfct_d = nc.declare_dram_parameter("bfct", [128, 1], f32, isOutput=False)
    out_d = nc.declare_dram_parameter("outT", [O, BL], f32, isOutput=True)

    with tile.TileContext(nc) as tc, ExitStack() as ctx:
        if tiny:
            pool = ctx.enter_context(tc.tile_pool(name="tp", bufs=1))
            t1 = pool.tile([128, BL], mdt, tag="t1")
            t2 = pool.tile([128, BL], f32, tag="t2")
            nc.sync.dma_start(t1[:, :], xprep_d[:, :, 0])
            nc.vector.tensor_copy(t2[:, :], t1[:, :])
            nc.sync.dma_start(out_d[:, :], t2[:, :])
            nc.compile()
            return nc

        const = ctx.enter_context(tc.tile_pool(name="const", bufs=1))
        work = ctx.enter_context(tc.tile_pool(name="work", bufs=4))
        # single-slot pool for ig: the WAR on the shared slot forces the
        # scheduler to finish stream A's c-update before starting stream
        # B's ig, keeping the DVE FIFO round-robin between the two layers.
        igp = ctx.enter_context(tc.tile_pool(name="igp", bufs=1))
        psum = ctx.enter_context(tc.tile_pool(name="psum", bufs=1, space="PSUM"))

        # ---- persistent SBUF residents ----
        # xall col = b*T + t
        xall = const.tile([128, BL * T], mdt, tag="xall")
        w0 = const.tile([128, G4], mdt, tag="w0")
        u0 = [const.tile([128, G4], mdt, tag=f"u0_{k}", name=f"u0_{k}")
              for k in range(KT)]
        w1 = [const.tile([128, G4], mdt, tag=f"w1_{k}", name=f"w1_{k}")
              for k in range(KT)]
        u1 = [const.tile([128, G4], mdt, tag=f"u1_{k}", name=f"u1_{k}")
              for k in range(KT)]
        wf = [const.tile([128, O], mdt, tag=f"wf_{k}", name=f"wf_{k}")
              for k in range(KT)]
        # bias matrices for the rank-1 bias matmuls: row r (partition r) =
        # bias of m-block r; block-one-hot rhs maps row m to col block m,
        # so ONE matmul adds every m-block's bias to a whole PSUM bank.
        ball0 = const.tile([128, 128], mdt, tag="ball0")
        ball1 = const.tile([128, 128], mdt, tag="ball1")
        onehot = const.tile([128, BKCOLS], mdt, tag="onehot")
        bfct = const.tile([128, 1], f32, tag="bfct")
        # h0 stream for one chunk, parity-major: col = k*256 + (s%4)*64
        #   + (s//4)*4 + b
        h0t = const.tile([128, KT * CHUNK * BL], mdt, tag="h0t")
        # recurrent state czg = [c (8) | zg staging (8)], col = k*BL + b;
        # keeping c and zg adjacent lets ONE tensor_tensor compute both
        # f*c and i*zg (gate layout is f,i,o,g).
        czg0 = const.tile([128, 2 * KT * BL], f32, tag="czg0")
        czg1 = const.tile([128, 2 * KT * BL], f32, tag="czg1")
        h1 = const.tile([128, KT * BL], mdt, tag="h1")

        # ---- PSUM: z for each layer, 4 banks, bank = step parity ----
        # col = (s%4)*BKCOLS + m*TG*BL + (s//4)*BL + b
        zl0 = psum.tile([128, NP * BKCOLS], f32, tag="zl0")
        zl1 = psum.tile([128, NP * BKCOLS], f32, tag="zl1")

        nc.sync.dma_start(xall[:, :].rearrange("p (b t) -> p b t", b=BL),
                          xprep_d[:, :, :])
        nc.sync.dma_start(w0[:, :], w0_d[:, :])
        for k in range(KT):
            sl = slice(k * 128, (k + 1) * 128)
            nc.sync.dma_start(u0[k][:, :], u0_d[sl, :])
            nc.sync.dma_start(w1[k][:, :], w1_d[sl, :])
            nc.sync.dma_start(u1[k][:, :], u1_d[sl, :])
            nc.sync.dma_start(wf[k][:, :], wfc_d[sl, :])
        nc.sync.dma_start(ball0[:, :], b0r_d[:, :])
        nc.sync.dma_start(ball1[:, :], b1r_d[:, :])
        nc.sync.dma_start(bfct[:, :], bfct_d[:, :])
        nc.sync.dma_start(onehot[:, :], oneh_d[:, :])
        nc.vector.memset(h0t[:, :], 0.0)
        nc.vector.memset(czg0[:, :], 0.0)
        nc.vector.memset(czg1[:, :], 0.0)
        nc.vector.memset(h1[:, :], 0.0)

        # z bank layout is tg-major: col = pr*BKCOLS + tg*(NM*BL) + m*BL + b
        # so one step's z is 32 CONTIGUOUS fp32 (i,f,o = 24, g = 8).
        # h0 layout: col = k*CHUNK*BL + pr*TG*BL + tg*BL + b
        h0v = h0t[:, :].rearrange("p (k pr tg b) -> p k pr tg b",
                                  k=KT, pr=NP, tg=TG)

        SW = NM * BL          # 32 z cols per step

        def zstep(zl, s):
            """offset of step s's z block (32 cols)."""
            return (s % NP) * BKCOLS + (s // NP) * SW

        def zcols(zl, s, m):
            """z AP for step s, m-block m: [128, BL]."""
            base = zstep(zl, s) + m * BL
            return zl[:, base:base + BL]

        def h0cols(s, k):
            return h0v[:, k, s % NP, s // NP]

        def h0ap_write(s):
            """[128, KT, BL] write AP for step s's h in h0t."""
            return h0v[:, :, s % NP, s // NP]

        # m-block order: g (6,7) first, then i (0,1), f (2,3), o (4,5)
        M_ORDER = [6, 7, 0, 1, 2, 3, 4, 5]

        def lstm_step(s, uw, zl, cst, h_rhs_fn, h_out_ap):
            S = KT * BL
            for m in M_ORDER:
                for k in range(KT):
                    nc.tensor.matmul(zcols(zl, s, m),
                                     lhsT=uw[k][:, m * 128:(m + 1) * 128],
                                     rhs=h_rhs_fn(k),
                                     start=False, stop=(k == KT - 1))
            zb = zstep(zl, s)
            # one sigmoid covers f,i,o; g is consumed straight from PSUM
            sg = work.tile([128, 3 * S], f32, tag="sg")
            nc.scalar.activation(sg[:, :], zl[:, zb:zb + 3 * S], AF.Sigmoid)
            # DVE: c = f*c + i*g ; h = o*c
            ig = igp.tile([128, S], f32, tag="ig")
            nc.vector.tensor_tensor(cst[:, 0:S], sg[:, 0:S], cst[:, 0:S],
                                    ALU.mult)
            nc.vector.tensor_tensor(ig[:, :], sg[:, S:2 * S],
                                    zl[:, zb + 3 * S:zb + 4 * S], ALU.mult)
            nc.vector.tensor_tensor(cst[:, 0:S], cst[:, 0:S], ig[:, :],
                                    ALU.add)
            nc.vector.tensor_tensor(
                h_out_ap,
                sg[:, 2 * S:3 * S].rearrange("p (k b) -> p k b", k=KT),
                cst[:, 0:S].rearrange("p (k b) -> p k b", k=KT), ALU.mult)

        def proj_bias(zl, ball):
            """z += bias: one N=512 matmul per bank adds every m-block's
            bias (ball row m) to its col block via the block-one-hot rhs."""
            for p in range(NP):
                nc.tensor.matmul(
                    zl[:, p * BKCOLS:(p + 1) * BKCOLS],
                    lhsT=ball[:, :], rhs=onehot[:, :],
                    start=False, stop=False)

        # proj-output view [p, pr, tg, m, b] (strided per m-block)
        zp0 = zl0[:, :].rearrange("p (pr tg m b) -> p pr tg m b",
                                  pr=NP, tg=TG, m=NM)
        zp1 = zl1[:, :].rearrange("p (pr tg m b) -> p pr tg m b",
                                  pr=NP, tg=TG, m=NM)

        def proj_l0(iv):
            """xw0 for chunk at t0=iv into zl0 (start=True clears banks)."""
            xq = work.tile([128, BL * CHUNK], mdt, tag="xq")
            # xq col = (t%4)*64 + (t//4)*4 + b  <- xall[b*T + iv + p + 4*tg]
            src = xall[:, :].rearrange("p (b t) -> p b t", b=BL)
            nc.vector.tensor_copy(
                xq[:, :].rearrange("p (pr tg b) -> p pr tg b", pr=NP, tg=TG),
                src[:, :, bass.ds(iv, CHUNK)].rearrange(
                    "p b (tg pr) -> p pr tg b", pr=NP),
            )
            for m in range(NM):
                for p in range(NP):
                    nc.tensor.matmul(
                        zp0[:, p, :, m],
                        lhsT=w0[:, m * 128:(m + 1) * 128],
                        rhs=xq[:, p * TG * BL:(p + 1) * TG * BL],
                        start=(m == 0), stop=False)
            proj_bias(zl0, ball0)

        def proj_l1():
            """xw1 = W1 @ h0(prev chunk) into zl1."""
            for m in range(NM):
                for p in range(NP):
                    for k in range(KT):
                        nc.tensor.matmul(
                            zp1[:, p, :, m],
                            lhsT=w1[k][:, m * 128:(m + 1) * 128],
                            rhs=h0t[:, k * CHUNK * BL + p * TG * BL:
                                    k * CHUNK * BL + (p + 1) * TG * BL],
                            start=(m == 0 and k == 0), stop=False)
            proj_bias(zl1, ball1)

        def l0_step(s):
            lstm_step(s, u0, zl0, czg0,
                      lambda k, _s=s: h0cols((_s - 1) % CHUNK, k),
                      h0ap_write(s))

        def l1_step(s):
            lstm_step(s, u1, zl1, czg1,
                      lambda k: h1[:, k * BL:(k + 1) * BL],
                      h1[:, :].rearrange("p (k b) -> p k b", k=KT))

        # ---- prologue: L0 chunk 0 ----
        proj_l0(0)
        for s in range(CHUNK):
            l0_step(s)

        # ---- main loop: L0 chunk i (t0=iv), L1 chunk i-1 ----
        from concourse.engine_type import EngineType
        with tc.For_i(CHUNK, T, CHUNK,
                      hint_engines=(EngineType.PE, EngineType.DVE,
                                    EngineType.Activation)) as iv:
            proj_l1()
            proj_l0(iv)
            for s in range(CHUNK):
                l0_step(s)
                l1_step(s)

        # ---- epilogue: L1 chunk 31, FC head ----
        proj_l1()
        for s in range(CHUNK):
            l1_step(s)

        psf = zl0[:, 0:BL]
        for k in range(KT):
            nc.tensor.matmul(psf, lhsT=wf[k][:, :],
                             rhs=h1[:, k * BL:(k + 1) * BL],
                             start=(k == 0), stop=(k == KT - 1))
        oT = work.tile([128, BL], f32, tag="oT")
        nc.scalar.activation(oT[:, :], psf, AF.Identity, bias=bfct[:, 0:1])
        nc.sync.dma_start(out_d[:, :], oT[:, :])

    nc.compile()
    return nc


def _get_compiled():
    if "main" not in _cache:
        _cache["main"] = _build()
    return _cache["main"]


def _ballmat(b, perm, mdt):
    m = np.zeros((128, 128), np.float32)
    m[0:NM, :] = np.asarray(b, np.float32)[perm].reshape(NM, 128)
    return np.ascontiguousarray(m.astype(mdt))


def _onehot(mdt):
    # z bank layout is tg-major: col = tg*(NM*BL) + m*BL + b -> row m hot
    # wherever (col % (NM*BL)) // BL == m
    m = np.zeros((128, BKCOLS), np.float32)
    cols = np.arange(BKCOLS)
    m[(cols % (NM * BL)) // BL, cols] = 1.0
    return np.ascontiguousarray(m.astype(mdt))


def _in_maps(input_seq, W0, U0, b0, W1, U1, b1, Wfc, bfc):
    mdt = _np_mmdt()
    x = np.asarray(input_seq, dtype=np.float32)
    # reorder gate blocks (i,f,g,o) -> (f,i,o,g)
    perm = np.concatenate([np.arange(H, 2 * H),
                           np.arange(0, H),
                           np.arange(3 * H, 4 * H),
                           np.arange(2 * H, 3 * H)])

    def gp(w):
        return np.ascontiguousarray(
            np.asarray(w, np.float32)[..., perm].astype(mdt))

    shared = {
        "w0": gp(W0),
        "u0": gp(U0),
        "w1": gp(W1),
        "u1": gp(U1),
        "wfc": np.ascontiguousarray(np.asarray(Wfc, np.float32).astype(mdt)),
        "b0r": _ballmat(b0, perm, mdt),
        "b1r": _ballmat(b1, perm, mdt),
        "oneh": _onehot(mdt),
        "bfct": np.ascontiguousarray(
            np.asarray(bfc, np.float32).reshape(1, 128).T),
    }
    in_maps = []
    for c in range(NCORES):
        xs = x[c * BL:(c + 1) * BL]                       # [BL, T, I]
        xp = np.ascontiguousarray(xs.transpose(2, 0, 1).astype(mdt))
        m = dict(shared)
        m["xprep"] = xp
        in_maps.append(m)
    return in_maps


def _assemble(res):
    out = np.empty((B, 1, O), np.float32)
    for c in range(NCORES):
        out[c * BL:(c + 1) * BL, 0, :] = res.results[c]["outT"].T
    return out


def _run(nc, inputs):
    from concourse.bass_utils import run_bass_kernel_spmd
    in_maps = _in_maps(**inputs)
    res = run_bass_kernel_spmd(nc, in_maps, list(range(NCORES)))
    return _assemble(res)


def kernel(input_seq, W0, U0, b0, W1, U1, b1, Wfc, bfc):
    nc = _get_compiled()
    return _run(nc, dict(input_seq=input_seq, W0=W0, U0=U0, b0=b0, W1=W1,
                         U1=U1, b1=b1, Wfc=Wfc, bfc=bfc))
